# revision 21
# baseline (speedup 1.0000x reference)
"""ChebyNet (K=3, 2 layers) forward on 8 Trainium2 NeuronCores.

Strategy: node sharding. Each core owns 1280 padded rows (10000 -> 10240).
The sparse propagation  L = -D^-1/2 A D^-1/2  is computed as a dense matmul
against the transposed adjacency-count matrix AT[s, d], held SBUF-resident in
fp8e4m3 (counts are small ints -> exact). Features move in bf16, accumulation
in fp32 PSUM, diagonal scalings as per-partition scalar multiplies.

Both layers are restructured using linearity of L, so each hop propagates the
minimum column count:

  Layer 1:  h = relu( x(W10-W12) + L( x W11 + L(x 2W12) ) + b1 )
  Layer 2:  out = h(W20-W22) + L( h W21 + L(h 2W22) ) + b2

Pipelined schedule: the four propagation hops form one continuous PE matmul
stream. Between hops the dis-scaled features are AllGathered in small pieces
(<=64KB input -> Mesh algorithm, ~7us each) that are emitted as soon as the
producing output blocks close, so collective latency hides under the next
hop's matmuls. Two SBUF u-buffer sets ping-pong between rounds so reloads
overlap compute. Hop A chases the adjacency-chunk DMAs (5 PSUM groups open
across the whole wave); hops C/D run transposed per dst-chunk pass so their
staging is staggered too.
"""

import sys

for _p in ("/opt/trn_rl_repo", "/root/.axon_site", "/root/.axon_site/_ro/trn_rl_repo",
           "/root/.axon_site/_ro/pypackages"):
    if _p not in sys.path:
        sys.path.append(_p)

import numpy as np
import ml_dtypes

import concourse.bacc as bacc
import concourse.tile as tile
from concourse import bass, mybir
from concourse.bass_utils import run_bass_kernel_spmd
from concourse.masks import make_identity
from concourse import bass_utils as _bu

# walrus passes --enable-ldw-opt=false by default; this kernel's prop sweep
# is LDWEIGHTS-bound (one 128-col fp8 stationary tile per matmul), so flip
# it on for this kernel's compile.
if not getattr(_bu, "_ldw_patch", False):
    _orig_run_command = _bu.run_command

    def _run_command_ldw(argv, **kw):
        argv = list(argv)
        return _orig_run_command(argv, **kw)

    _bu.run_command = _run_command_ldw
    _bu._ldw_patch = True

# problem constants (hardcoded per harness contract)
N, E, IN, HID, OUT, K = 10000, 320000, 256, 256, 128, 3
CORES = 8
NP = 10240          # padded node count
RPC = NP // CORES   # rows per core = 1280
MB = RPC // 128     # M-blocks per core = 10
KT = NP // 128      # K-tiles = 80
KTE = KT - 1        # kt=79 is all-pad src rows (zero block) -> skipped
F = IN              # layer-1 prop width = 256
P = 128
CH = 8              # at DMA chunks
KPC = KT // CH      # kts per at chunk = 10
HCH = 16            # xT DMA chunks
HWC = NP // HCH     # xT chunk cols = 640 (5 kts)

FP8 = mybir.dt.float8e4
BF16 = mybir.dt.bfloat16
F32 = mybir.dt.float32

_STATE = {}


def _build():
    nc = bacc.Bacc("TRN2", target_bir_lowering=False, debug=False, num_devices=CORES)

    at_d = nc.dram_tensor("at", [P, KT * RPC], FP8, kind="ExternalInput")
    xoT_d = nc.dram_tensor("xoT", [F, RPC], BF16, kind="ExternalInput")
    xT_d = nc.dram_tensor("xT", [2, P, NP], BF16, kind="ExternalInput")
    disf_d = nc.dram_tensor("disf", [P, KT], F32, kind="ExternalInput")
    diso_d = nc.dram_tensor("diso", [P, MB], F32, kind="ExternalInput")
    ndiso_d = nc.dram_tensor("ndiso", [P, MB], F32, kind="ExternalInput")
    dd_d = nc.dram_tensor("dd", [P, MB], F32, kind="ExternalInput")
    # w1x = [W1[0]-W1[2], W1[1], 2*W1[2]], w2x likewise for W2
    w1x_d = nc.dram_tensor("w1x", [K, IN, HID], BF16, kind="ExternalInput")
    w2x_d = nc.dram_tensor("w2x", [K, HID, OUT], BF16, kind="ExternalInput")
    b1r_d = nc.dram_tensor("b1r", [P, HID], BF16, kind="ExternalInput")
    b2r_d = nc.dram_tensor("b2r", [P, OUT], BF16, kind="ExternalInput")
    out_d = nc.dram_tensor("outo", [RPC, OUT], F32, kind="ExternalOutput")

    xoT_r = xoT_d.ap().rearrange("(c p) d -> c p d", p=P)

    with tile.TileContext(nc) as tc:
        with (
            tc.tile_pool(name="res", bufs=1) as res,
            tc.tile_pool(name="wrk", bufs=1) as wrk,
            tc.tile_pool(name="pprop", bufs=5, space="PSUM") as pprop,
            tc.tile_pool(name="pterm", bufs=2, space="PSUM") as pterm,
            tc.tile_pool(name="ptr", bufs=1, space="PSUM") as ptr,
            tc.tile_pool(name="dram", bufs=1, space="DRAM") as dram,
        ):
            # ---- small loads ----
            xoT_t = []
            for c in range(2):
                t = res.tile([P, RPC], BF16, tag="xoTsT", bufs=2, name=f"xoT{c}")
                nc.sync.dma_start(t[:], xoT_r[c])
                xoT_t.append(t)
            w1t = [[None, None] for _ in range(K)]
            for k in range(K):
                for c in range(2):
                    t = res.tile([P, HID], BF16, tag="wh", bufs=6, name=f"w1_{k}_{c}")
                    nc.sync.dma_start(t[:], w1x_d[k, c * P:(c + 1) * P, :])
                    w1t[k][c] = t
            w2t = [[None, None] for _ in range(K)]
            for k in range(K):
                for c in range(2):
                    t = res.tile([P, OUT], BF16, tag=f"w2_{k}_{c}", name=f"w2_{k}_{c}")
                    nc.sync.dma_start(t[:], w2x_d[k, c * P:(c + 1) * P, :])
                    w2t[k][c] = t
            diso = res.tile([P, MB], F32, name="diso")
            nc.sync.dma_start(diso[:], diso_d[:])
            ndiso = res.tile([P, MB], F32, name="ndiso")
            nc.sync.dma_start(ndiso[:], ndiso_d[:])
            dd = res.tile([P, MB], F32, name="dd")
            nc.sync.dma_start(dd[:], dd_d[:])
            b1r = res.tile([P, HID], BF16, name="b1r")
            nc.sync.dma_start(b1r[:], b1r_d[:])
            b2r = res.tile([P, OUT], BF16, name="b2r")
            nc.sync.dma_start(b2r[:], b2r_d[:])
            disf = res.tile([P, KT], F32, name="disf")
            nc.sync.dma_start(disf[:], disf_d[:])

            ident = res.tile([P, P], F32, name="ident")
            make_identity(nc, ident[:])
            idb = res.tile([P, P], BF16, name="idb")
            nc.vector.tensor_copy(idb[:], ident[:])

            # ---- tiny dummy collective: absorbs the one-time CC bootstrap ----
            dumi = dram.tile([P, 16], BF16, name="dumi")
            dumo = dram.tile([CORES * P, 16], BF16, name="dumo", addr_space="Shared")
            nc.sync.dma_start(dumi[:], xT_d[0, :, 0:16])
            nc.gpsimd.collective_compute(
                "AllGather", mybir.AluOpType.bypass,
                replica_groups=[list(range(CORES))],
                ins=[dumi[:].opt()], outs=[dumo[:].opt()],
            )

            # ---- bulk DMAs, interleaved in chunk groups so arrival is
            # ordered (xT for d2 first, then the at sub-chunks the wave-1
            # matmuls of that group consume). Sub-chunk DMAs spread each
            # group across the DMA queues for full aggregate bandwidth.
            atc = []
            xTc_t = [[None, None] for _ in range(HCH)]
            for c8 in range(CH):
                nkt = KPC if c8 < CH - 1 else KPC - 1
                t = res.tile([P, nkt * RPC], FP8, name=f"atc{c8}")
                atc.append(t)
                for hc in (2 * c8, 2 * c8 + 1):
                    for c in range(2):
                        xt = wrk.tile([P, HWC], BF16, tag=f"xTc{c}", bufs=2,
                                      name=f"xTc{hc}_{c}")
                        nc.sync.dma_start(xt[:], xT_d[c, :, hc * HWC:(hc + 1) * HWC])
                        xTc_t[hc][c] = xt
                base = c8 * KPC * RPC
                for s0 in range(0, nkt, 2):
                    sw = min(2, nkt - s0) * RPC
                    nc.sync.dma_start(t[:, s0 * RPC:s0 * RPC + sw],
                                      at_d[:, base + s0 * RPC:base + s0 * RPC + sw])

            def at_sl(kt, lo, w):
                c8, r = divmod(kt, KPC)
                base = r * RPC + lo
                return atc[c8][:, base:base + w]

            # persistent per-block tensors (bf16; tags shared across layers)
            d1z_t = [res.tile([P, F], BF16, tag="dz", bufs=MB, name=f"d1_{m}")
                     for m in range(MB)]
            hw_t = [res.tile([P, F], BF16, tag="ehw", bufs=MB, name=f"e0_{m}")
                    for m in range(MB)]  # holds e0 now, hw later (slot reuse)
            e0_t = hw_t

            # u tile sets: uA = hop A input (d2), then round-1 (hop C input);
            # uB = hop B input (s1), then round-2 (hop D input).
            uA = [res.tile([P, F], BF16, tag=f"uA{kt}", name=f"uA{kt}")
                  for kt in range(KTE)]
            uB = [res.tile([P, F], BF16, tag=f"uB{kt}", name=f"uB{kt}")
                  for kt in range(KTE)]

            def mm6(psum_ap, lhsTs, rhs_pair):
                nc.tensor.matmul(psum_ap, lhsTs[0], rhs_pair[0][:], start=True, stop=False)
                nc.tensor.matmul(psum_ap, lhsTs[1], rhs_pair[1][:], start=False, stop=True)

            # ---- d1 = x@W11, e0 = x@(W10-W12) from own-rows x^T ----
            xoT_sl = [[xoT_t[c][:, m * P:(m + 1) * P] for c in range(2)] for m in range(MB)]
            for mb in range(MB):
                dp = pterm.tile([P, F], F32, tag="tp", name=f"d1p_{mb}")
                mm6(dp[:], xoT_sl[mb], w1t[1])
                nc.vector.tensor_scalar_mul(d1z_t[mb][:], dp[:], diso[:, mb:mb + 1])
                e0p = pterm.tile([P, F], F32, tag="tp", name=f"e0p_{mb}")
                mm6(e0p[:], xoT_sl[mb], w1t[0])
                nc.vector.tensor_copy(e0_t[mb][:], e0p[:])

            # ---- hop A wave 1 (mbs 0-4) chases the at/xT DMA stream.
            # d2 for each chunk's kts is computed just ahead of the wave-1
            # matmuls that consume it.
            W1MB = 5
            pp_w1 = [pprop.tile([P, 512], F32, tag="pp", name=f"ppw{mb}")
                     for mb in range(W1MB)]
            for c8 in range(CH):
                for hc in (2 * c8, 2 * c8 + 1):
                    for m5 in range(5):
                        kt = hc * 5 + m5
                        if kt >= KTE:
                            continue
                        dp = pterm.tile([P, F], F32, tag="tp", name=f"d2f_{kt}")
                        mm6(dp[:], [xTc_t[hc][c][:, m5 * P:(m5 + 1) * P]
                                    for c in range(2)], w1t[2])
                        nc.vector.tensor_scalar_mul(uA[kt][:], dp[:],
                                                    disf[:, kt:kt + 1])
                for r in range(KPC):
                    kt = c8 * KPC + r
                    if kt >= KTE:
                        continue
                    for mb in range(W1MB):
                        nc.tensor.matmul(
                            pp_w1[mb][:, :F], at_sl(kt, mb * P, P), uA[kt][:],
                            start=(kt == 0), stop=(kt == KTE - 1),
                        )

            # AG pieces: 3 per round (CC stream allows ~2 in flight; fewer,
            # larger ops beat many small ones). mb-groups per piece:
            PC0 = [(0, 1, 2, 3, 4), (5, 6), (7, 8, 9)]      # round 0 (256-wide)
            PC1 = [(0, 1, 2, 3), (4, 5, 6, 7), (8, 9)]      # round 1 (128-wide)
            PC2 = [(8, 9), (0, 1, 2, 3), (4, 5, 6, 7)]      # round 2 (by C pass)

            def mk_ag(tag, pieces, w):
                ins = [dram.tile([len(ms) * P, w], BF16, name=f"agi{tag}_{i}")
                       for i, ms in enumerate(pieces)]
                outs = [dram.tile([CORES * len(ms) * P, w], BF16,
                                  name=f"ago{tag}_{i}", addr_space="Shared")
                        for i, ms in enumerate(pieces)]
                return ins, outs

            agi0_t, ago0_t = mk_ag("0", PC0, F)
            agi1_t, ago1_t = mk_ag("1", PC1, OUT)
            agi2_t, ago2_t = mk_ag("2", PC2, OUT)

            def piece_of(pieces, mb):
                for i, ms in enumerate(pieces):
                    if mb in ms:
                        return i, ms.index(mb)
                raise AssertionError

            def stage_piece(agi_list, pieces, mb, src_ap):
                i, idx = piece_of(pieces, mb)
                nc.sync.dma_start(agi_list[i][idx * P:(idx + 1) * P, :], src_ap)

            def emit_ag(iap, oap):
                nc.gpsimd.collective_compute(
                    "AllGather", mybir.AluOpType.bypass,
                    replica_groups=[list(range(CORES))],
                    ins=[iap.opt()], outs=[oap.opt()],
                )

            def reload_piece(ago_list, pieces, i, dst_set, w):
                ms = pieces[i]
                for c8 in range(CORES):
                    for idx, m in enumerate(ms):
                        kt = c8 * KPC + m
                        if kt >= KTE:
                            continue
                        base = (c8 * len(ms) + idx) * P
                        nc.sync.dma_start(dst_set[kt][:, :w],
                                          ago_list[i][base:base + P, :])

            def stage_s1(mb, pp_ap):
                sc = wrk.tile([P, F], BF16, tag="sc", bufs=2, name=f"scA_{mb}")
                nc.vector.tensor_scalar_mul(sc[:], pp_ap, dd[:, mb:mb + 1])
                nc.vector.tensor_add(sc[:], sc[:], d1z_t[mb][:])
                stage_piece(agi0_t, PC0, mb, sc[:])

            for mb in range(W1MB):
                stage_s1(mb, pp_w1[mb][:, :F])
            emit_ag(agi0_t[0][:], ago0_t[0][:])
            reload_piece(ago0_t, PC0, 0, uB, F)

            # ---- hop A wave 2 (mbs 5-9), piece AGs as their mb-groups close ----
            for mb in range(W1MB, MB):
                pp = pprop.tile([P, 512], F32, tag="pp", name=f"ppw2_{mb}")
                for kt in range(KTE):
                    nc.tensor.matmul(
                        pp[:, :F], at_sl(kt, mb * P, P), uA[kt][:],
                        start=(kt == 0), stop=(kt == KTE - 1),
                    )
                stage_s1(mb, pp[:, :F])
                if mb == 6:
                    emit_ag(agi0_t[1][:], ago0_t[1][:])
                    reload_piece(ago0_t, PC0, 1, uB, F)
                elif mb == 9:
                    emit_ag(agi0_t[2][:], ago0_t[2][:])
                    reload_piece(ago0_t, PC0, 2, uB, F)

            # ---- hop B + layer-2 feature matmuls, software-pipelined ----
            # kt consumption order matches round-0 piece arrival
            orderB = [c8 * KPC + m for ms in PC0 for m in ms for c8 in range(CORES)]
            orderB = [kt for kt in orderB if kt < KTE]

            pp_b = [None] * MB

            def post_B(m):
                h = wrk.tile([P, F], BF16, tag="h", bufs=2, name=f"h_{m}")
                nc.vector.tensor_scalar_mul(h[:], pp_b[m][:, :F], ndiso[:, m:m + 1])
                nc.vector.tensor_add(h[:], h[:], e0_t[m][:])
                nc.vector.tensor_add(h[:], h[:], b1r[:])
                nc.vector.tensor_scalar_max(h[:], h[:], 0.0)
                hT = []
                for c in range(2):
                    tps = ptr.tile([P, P], BF16, tag="tr", name=f"hTp_{m}_{c}")
                    nc.tensor.transpose(tps[:], h[:, c * P:(c + 1) * P], idb[:])
                    tb = res.tile([P, HID], BF16, tag="wh", bufs=6, name=f"hTs_{m}_{c}")
                    nc.vector.tensor_copy(tb[:, :P], tps[:])
                    hT.append(tb[:, :P])
                zp = pterm.tile([P, F], F32, tag="tp", name=f"z1p_{m}")
                mm6(zp[:, :OUT], hT, w2t[1])
                nc.vector.tensor_scalar_mul(d1z_t[m][:, :OUT], zp[:, :OUT],
                                            diso[:, m:m + 1])
                z2p = pterm.tile([P, F], F32, tag="tp", name=f"z2p_{m}")
                mm6(z2p[:, :OUT], hT, w2t[2])
                sc = wrk.tile([P, F], BF16, tag="sc", bufs=2, name=f"scB_{m}")
                nc.vector.tensor_scalar_mul(sc[:, :OUT], z2p[:, :OUT], diso[:, m:m + 1])
                stage_piece(agi1_t, PC1, m, sc[:, :OUT])
                hwp = pterm.tile([P, F], F32, tag="tp", name=f"hwp_{m}")
                mm6(hwp[:, :OUT], hT, w2t[0])
                nc.vector.tensor_copy(hw_t[m][:, :OUT], hwp[:, :OUT])
                for i, ms in enumerate(PC1):
                    if m == ms[-1]:
                        emit_ag(agi1_t[i][:], ago1_t[i][:])
                        reload_piece(ago1_t, PC1, i, uA, OUT)

            for mb in range(MB + 1):
                if mb < MB:
                    pp_b[mb] = pprop.tile([P, 512], F32, tag="pp", name=f"ppb_{mb}")
                    for kt in orderB:
                        nc.tensor.matmul(
                            pp_b[mb][:, :F], at_sl(kt, mb * P, P), uB[kt][:],
                            start=(kt == orderB[0]), stop=(kt == orderB[-1]),
                        )
                if mb > 0:
                    post_B(mb - 1)

            # ---- hops C and D: transposed, one pass per dst chunk so staging
            # is staggered. 256-wide chunk first in C (earliest AG2 piece);
            # 512-wide chunks first in D (256 chunk's input arrives last).
            passC = [(1024, 256), (0, 512), (512, 512)]
            passD = [(0, 512), (512, 512), (1024, 256)]
            orderC = [c8 * KPC + m for ms in PC1 for m in ms for c8 in range(CORES)]
            orderC = [kt for kt in orderC if kt < KTE]
            orderD = [c8 * KPC + m for ms in PC2 for m in ms for c8 in range(CORES)]
            orderD = [kt for kt in orderD if kt < KTE]

            ppc = [None] * 3

            def prop_T(pi, off, w, u_set, order, tagn):
                ppc[pi] = pprop.tile([P, 512], F32, tag="pp", name=f"{tagn}_{pi}")
                for j, kt in enumerate(order):
                    nc.tensor.matmul(
                        ppc[pi][:, :w], u_set[kt][:, :OUT], at_sl(kt, off, w),
                        start=(j == 0), stop=(j == len(order) - 1),
                    )

            def post_C(pi):
                off, w = passC[pi]
                sT = res.tile([P, 512], F32, tag="xoTsT", bufs=2, name=f"sTc_{pi}")
                nc.vector.tensor_copy(sT[:, :w], ppc[pi][:, :w])
                for i in range(w // P):
                    mb = off // P + i
                    tps = ptr.tile([P, P], F32, tag="tr", name=f"trC_{mb}")
                    nc.tensor.transpose(tps[:], sT[:, i * P:(i + 1) * P], ident[:])
                    sc = wrk.tile([P, F], BF16, tag="sc", bufs=2, name=f"scC_{mb}")
                    nc.vector.tensor_scalar_mul(sc[:, :OUT], tps[:], dd[:, mb:mb + 1])
                    nc.vector.tensor_add(sc[:, :OUT], sc[:, :OUT], d1z_t[mb][:, :OUT])
                    stage_piece(agi2_t, PC2, mb, sc[:, :OUT])
                    for i2, ms in enumerate(PC2):
                        if mb == ms[-1]:
                            emit_ag(agi2_t[i2][:], ago2_t[i2][:])
                            reload_piece(ago2_t, PC2, i2, uB, OUT)

            for pi in range(3):
                off, w = passC[pi]
                prop_T(pi, off, w, uA, orderC, "ppc")
                post_C(pi)

            def post_D(pi):
                off, w = passD[pi]
                sT = res.tile([P, 512], F32, tag="xoTsT", bufs=2, name=f"sTd_{pi}")
                nc.vector.tensor_copy(sT[:, :w], ppc[pi][:, :w])
                for i in range(w // P):
                    mb = off // P + i
                    tps = ptr.tile([P, P], F32, tag="tr", name=f"trD_{mb}")
                    nc.tensor.transpose(tps[:], sT[:, i * P:(i + 1) * P], ident[:])
                    oacc = wrk.tile([P, OUT], F32, tag="sf", bufs=2, name=f"oacc_{mb}")
                    nc.vector.tensor_scalar_mul(oacc[:], tps[:], ndiso[:, mb:mb + 1])
                    nc.vector.tensor_add(oacc[:], oacc[:], hw_t[mb][:, :OUT])
                    nc.vector.tensor_add(oacc[:], oacc[:], b2r[:])
                    nc.sync.dma_start(out_d[mb * P:(mb + 1) * P, :], oacc[:])

            for pi in range(3):
                off, w = passD[pi]
                prop_T(pi, off, w, uB, orderD, "ppd")
                post_D(pi)

    nc.compile()
    return nc


def _prepare_inputs(x, edge, W1, b1, W2, b2):
    x = np.asarray(x, np.float32)
    edge = np.asarray(edge)
    W1 = np.asarray(W1, np.float32)
    b1 = np.asarray(b1, np.float32)
    W2 = np.asarray(W2, np.float32)
    b2 = np.asarray(b2, np.float32)
    src = edge[0].astype(np.int64)
    dst = edge[1].astype(np.int64)

    deg = np.bincount(dst, minlength=N).astype(np.float32)
    dis = np.where(deg > 0, 1.0 / np.sqrt(np.maximum(deg, 1.0)), 0.0).astype(np.float32)

    # dense transposed adjacency counts AT[s, d]
    flat = src * NP + dst
    uniq, cnt = np.unique(flat, return_counts=True)
    at8 = np.zeros(NP * NP, dtype=ml_dtypes.float8_e4m3)
    at8[uniq] = cnt.astype(ml_dtypes.float8_e4m3)
    at8 = at8.reshape(NP, NP)

    dis_pad = np.zeros(NP, np.float32)
    dis_pad[:N] = dis
    x_pad = np.zeros((NP, F), np.float32)
    x_pad[:N] = x

    w1x = np.stack([W1[0] - W1[2], W1[1], 2.0 * W1[2]]).astype(ml_dtypes.bfloat16)
    w2x = np.stack([W2[0] - W2[2], W2[1], 2.0 * W2[2]]).astype(ml_dtypes.bfloat16)
    b1r = np.broadcast_to(b1, (P, HID)).astype(ml_dtypes.bfloat16).copy()
    b2r = np.broadcast_to(b2, (P, OUT)).astype(ml_dtypes.bfloat16).copy()

    xTb = np.ascontiguousarray(x_pad.T).astype(ml_dtypes.bfloat16).reshape(2, P, NP)
    disf_h = np.ascontiguousarray(dis_pad.reshape(KT, P).T)
    in_maps = []
    for c in range(CORES):
        rows = slice(c * RPC, (c + 1) * RPC)
        dv = dis_pad[rows]
        atc = np.ascontiguousarray(
            at8[:, rows].reshape(KT, P, RPC).transpose(1, 0, 2).reshape(P, KT * RPC))
        m = {
            "at": atc,
            "xoT": np.ascontiguousarray(x_pad[rows].T).astype(ml_dtypes.bfloat16),
            "xT": xTb,
            "disf": disf_h,
            "diso": np.ascontiguousarray(dv.reshape(MB, P).T),
            "ndiso": np.ascontiguousarray((-dv).reshape(MB, P).T),
            "dd": np.ascontiguousarray((-dv * dv).reshape(MB, P).T),
            "w1x": w1x,
            "w2x": w2x,
            "b1r": b1r,
            "b2r": b2r,
        }
        in_maps.append(m)
    return in_maps


def _run(in_maps, trace=False, **kw):
    if "nc" not in _STATE:
        _STATE["nc"] = _build()
    r = run_bass_kernel_spmd(_STATE["nc"], in_maps, core_ids=list(range(CORES)),
                             trace=trace, **kw)
    out = np.concatenate([r.results[c]["outo"] for c in range(CORES)], axis=0)
    return out[:N], r


def kernel(**inputs) -> np.ndarray:
    in_maps = _prepare_inputs(**inputs)
    out, _ = _run(in_maps)
    return out


# revision 23
# speedup vs baseline: 1.0134x; 1.0134x over previous
"""ChebyNet (K=3, 2 layers) forward on 8 Trainium2 NeuronCores.

Strategy: node sharding. Each core owns 1280 padded rows (10000 -> 10240).
The sparse propagation  L = -D^-1/2 A D^-1/2  is computed as a dense matmul
against the transposed adjacency-count matrix AT[s, d], held SBUF-resident in
fp8e4m3 (counts are small ints -> exact). Features move in bf16, accumulation
in fp32 PSUM, diagonal scalings as per-partition scalar multiplies.

Both layers are restructured using linearity of L, so each hop propagates the
minimum column count:

  Layer 1:  h = relu( x(W10-W12) + L( x W11 + L(x 2W12) ) + b1 )
  Layer 2:  out = h(W20-W22) + L( h W21 + L(h 2W22) ) + b2

Pipelined schedule: the four propagation hops form one continuous PE matmul
stream. Between hops the dis-scaled features are AllGathered in small pieces
(<=64KB input -> Mesh algorithm, ~7us each) that are emitted as soon as the
producing output blocks close, so collective latency hides under the next
hop's matmuls. Two SBUF u-buffer sets ping-pong between rounds so reloads
overlap compute. Hop A chases the adjacency-chunk DMAs (5 PSUM groups open
across the whole wave); hops C/D run transposed per dst-chunk pass so their
staging is staggered too.
"""

import sys

for _p in ("/opt/trn_rl_repo", "/root/.axon_site", "/root/.axon_site/_ro/trn_rl_repo",
           "/root/.axon_site/_ro/pypackages"):
    if _p not in sys.path:
        sys.path.append(_p)

import numpy as np
import ml_dtypes

import concourse.bacc as bacc
import concourse.tile as tile
from concourse import bass, mybir
from concourse.bass_utils import run_bass_kernel_spmd
from concourse.masks import make_identity
from concourse import bass_utils as _bu

# walrus passes --enable-ldw-opt=false by default; this kernel's prop sweep
# is LDWEIGHTS-bound (one 128-col fp8 stationary tile per matmul), so flip
# it on for this kernel's compile.
if not getattr(_bu, "_ldw_patch", False):
    _orig_run_command = _bu.run_command

    def _run_command_ldw(argv, **kw):
        argv = list(argv)
        return _orig_run_command(argv, **kw)

    _bu.run_command = _run_command_ldw
    _bu._ldw_patch = True

# problem constants (hardcoded per harness contract)
N, E, IN, HID, OUT, K = 10000, 320000, 256, 256, 128, 3
CORES = 8
NP = 10240          # padded node count
RPC = NP // CORES   # rows per core = 1280
MB = RPC // 128     # M-blocks per core = 10
KT = NP // 128      # K-tiles = 80
KTE = KT - 1        # kt=79 is all-pad src rows (zero block) -> skipped
F = IN              # layer-1 prop width = 256
P = 128
CH = 8              # at DMA chunks
KPC = KT // CH      # kts per at chunk = 10
HCH = 16            # xT DMA chunks
HWC = NP // HCH     # xT chunk cols = 640 (5 kts)

FP8 = mybir.dt.float8e4
BF16 = mybir.dt.bfloat16
F32 = mybir.dt.float32

_STATE = {}


def _build():
    nc = bacc.Bacc("TRN2", target_bir_lowering=False, debug=False, num_devices=CORES)

    at_d = nc.dram_tensor("at", [P, KT * RPC], FP8, kind="ExternalInput")
    xoT_d = nc.dram_tensor("xoT", [F, RPC], BF16, kind="ExternalInput")
    xT_d = nc.dram_tensor("xT", [2, P, NP], BF16, kind="ExternalInput")
    disf_d = nc.dram_tensor("disf", [P, KT], F32, kind="ExternalInput")
    diso_d = nc.dram_tensor("diso", [P, MB], F32, kind="ExternalInput")
    ndiso_d = nc.dram_tensor("ndiso", [P, MB], F32, kind="ExternalInput")
    dd_d = nc.dram_tensor("dd", [P, MB], F32, kind="ExternalInput")
    # w1x = [W1[0]-W1[2], W1[1], 2*W1[2]], w2x likewise for W2
    w1x_d = nc.dram_tensor("w1x", [K, IN, HID], BF16, kind="ExternalInput")
    w2x_d = nc.dram_tensor("w2x", [K, HID, OUT], BF16, kind="ExternalInput")
    b1r_d = nc.dram_tensor("b1r", [P, HID], BF16, kind="ExternalInput")
    b2r_d = nc.dram_tensor("b2r", [P, OUT], BF16, kind="ExternalInput")
    out_d = nc.dram_tensor("outo", [RPC, OUT], F32, kind="ExternalOutput")

    xoT_r = xoT_d.ap().rearrange("(c p) d -> c p d", p=P)

    with tile.TileContext(nc) as tc:
        with (
            tc.tile_pool(name="res", bufs=1) as res,
            tc.tile_pool(name="wrk", bufs=1) as wrk,
            tc.tile_pool(name="pprop", bufs=5, space="PSUM") as pprop,
            tc.tile_pool(name="pterm", bufs=2, space="PSUM") as pterm,
            tc.tile_pool(name="ptr", bufs=1, space="PSUM") as ptr,
            tc.tile_pool(name="dram", bufs=1, space="DRAM") as dram,
        ):
            # ---- small loads ----
            xoT_t = []
            for c in range(2):
                t = res.tile([P, RPC], BF16, tag="xoTsT", bufs=2, name=f"xoT{c}")
                nc.sync.dma_start(t[:], xoT_r[c])
                xoT_t.append(t)
            w1t = [[None, None] for _ in range(K)]
            for k in range(K):
                for c in range(2):
                    t = res.tile([P, HID], BF16, tag="wh", bufs=6, name=f"w1_{k}_{c}")
                    nc.sync.dma_start(t[:], w1x_d[k, c * P:(c + 1) * P, :])
                    w1t[k][c] = t
            w2t = [[None, None] for _ in range(K)]
            for k in range(K):
                for c in range(2):
                    t = res.tile([P, OUT], BF16, tag=f"w2_{k}_{c}", name=f"w2_{k}_{c}")
                    nc.sync.dma_start(t[:], w2x_d[k, c * P:(c + 1) * P, :])
                    w2t[k][c] = t
            diso = res.tile([P, MB], F32, name="diso")
            nc.sync.dma_start(diso[:], diso_d[:])
            ndiso = res.tile([P, MB], F32, name="ndiso")
            nc.sync.dma_start(ndiso[:], ndiso_d[:])
            dd = res.tile([P, MB], F32, name="dd")
            nc.sync.dma_start(dd[:], dd_d[:])
            b1r = res.tile([P, HID], BF16, name="b1r")
            nc.sync.dma_start(b1r[:], b1r_d[:])
            b2r = res.tile([P, OUT], BF16, name="b2r")
            nc.sync.dma_start(b2r[:], b2r_d[:])
            disf = res.tile([P, KT], F32, name="disf")
            nc.sync.dma_start(disf[:], disf_d[:])

            ident = res.tile([P, P], F32, name="ident")
            make_identity(nc, ident[:])
            idb = res.tile([P, P], BF16, name="idb")
            nc.vector.tensor_copy(idb[:], ident[:])

            # ---- tiny dummy collective: absorbs the one-time CC bootstrap ----
            dumi = dram.tile([P, 16], BF16, name="dumi")
            dumo = dram.tile([CORES * P, 16], BF16, name="dumo", addr_space="Shared")
            nc.sync.dma_start(dumi[:], xT_d[0, :, 0:16])
            nc.gpsimd.collective_compute(
                "AllGather", mybir.AluOpType.bypass,
                replica_groups=[list(range(CORES))],
                ins=[dumi[:].opt()], outs=[dumo[:].opt()],
            )

            # ---- bulk DMAs, interleaved in chunk groups so arrival is
            # ordered (xT for d2 first, then the at sub-chunks the wave-1
            # matmuls of that group consume). Sub-chunk DMAs spread each
            # group across the DMA queues for full aggregate bandwidth.
            atc = []
            xTc_t = [[None, None] for _ in range(HCH)]
            for c8 in range(CH):
                nkt = KPC if c8 < CH - 1 else KPC - 1
                t = res.tile([P, nkt * RPC], FP8, name=f"atc{c8}")
                atc.append(t)
                for hc in (2 * c8, 2 * c8 + 1):
                    for c in range(2):
                        xt = wrk.tile([P, HWC], BF16, tag=f"xTc{c}", bufs=2,
                                      name=f"xTc{hc}_{c}")
                        nc.sync.dma_start(xt[:], xT_d[c, :, hc * HWC:(hc + 1) * HWC])
                        xTc_t[hc][c] = xt
                base = c8 * KPC * RPC
                for s0 in range(0, nkt, 2):
                    sw = min(2, nkt - s0) * RPC
                    nc.sync.dma_start(t[:, s0 * RPC:s0 * RPC + sw],
                                      at_d[:, base + s0 * RPC:base + s0 * RPC + sw])

            def at_sl(kt, lo, w):
                c8, r = divmod(kt, KPC)
                base = r * RPC + lo
                return atc[c8][:, base:base + w]

            # persistent per-block tensors (bf16; tags shared across layers)
            d1z_t = [res.tile([P, F], BF16, tag="dz", bufs=MB, name=f"d1_{m}")
                     for m in range(MB)]
            hw_t = [res.tile([P, F], BF16, tag="ehw", bufs=MB, name=f"e0_{m}")
                    for m in range(MB)]  # holds e0 now, hw later (slot reuse)
            e0_t = hw_t

            # u tile sets: uA = hop A input (d2), then round-1 (hop C input);
            # uB = hop B input (s1), then round-2 (hop D input).
            uA = [res.tile([P, F], BF16, tag=f"uA{kt}", name=f"uA{kt}")
                  for kt in range(KTE)]
            uB = [res.tile([P, F], BF16, tag=f"uB{kt}", name=f"uB{kt}")
                  for kt in range(KTE)]

            def mm6(psum_ap, lhsTs, rhs_pair):
                nc.tensor.matmul(psum_ap, lhsTs[0], rhs_pair[0][:], start=True, stop=False)
                nc.tensor.matmul(psum_ap, lhsTs[1], rhs_pair[1][:], start=False, stop=True)

            # ---- d1 = x@W11, e0 = x@(W10-W12) from own-rows x^T ----
            xoT_sl = [[xoT_t[c][:, m * P:(m + 1) * P] for c in range(2)] for m in range(MB)]
            for mb in range(MB):
                dp = pterm.tile([P, F], F32, tag="tp", name=f"d1p_{mb}")
                mm6(dp[:], xoT_sl[mb], w1t[1])
                nc.vector.tensor_scalar_mul(d1z_t[mb][:], dp[:], diso[:, mb:mb + 1])
                e0p = pterm.tile([P, F], F32, tag="tp", name=f"e0p_{mb}")
                mm6(e0p[:], xoT_sl[mb], w1t[0])
                nc.vector.tensor_copy(e0_t[mb][:], e0p[:])

            # ---- hop A wave 1 (mbs 0-4) chases the at/xT DMA stream.
            # d2 for each chunk's kts is computed just ahead of the wave-1
            # matmuls that consume it.
            W1MB = 5
            pp_w1 = [pprop.tile([P, 512], F32, tag="pp", name=f"ppw{mb}")
                     for mb in range(W1MB)]
            for c8 in range(CH):
                for hc in (2 * c8, 2 * c8 + 1):
                    for m5 in range(5):
                        kt = hc * 5 + m5
                        if kt >= KTE:
                            continue
                        dp = pterm.tile([P, F], F32, tag="tp", name=f"d2f_{kt}")
                        mm6(dp[:], [xTc_t[hc][c][:, m5 * P:(m5 + 1) * P]
                                    for c in range(2)], w1t[2])
                        nc.vector.tensor_scalar_mul(uA[kt][:], dp[:],
                                                    disf[:, kt:kt + 1])
                for r in range(KPC):
                    kt = c8 * KPC + r
                    if kt >= KTE:
                        continue
                    for mb in range(W1MB):
                        nc.tensor.matmul(
                            pp_w1[mb][:, :F], at_sl(kt, mb * P, P), uA[kt][:],
                            start=(kt == 0), stop=(kt == KTE - 1),
                        )

            # AG pieces: 3 per round (CC stream allows ~2 in flight; fewer,
            # larger ops beat many small ones). mb-groups per piece:
            PC0 = [(0, 1, 2, 3, 4), (5, 6, 7), (8, 9)]      # round 0 (256-wide)
            PC1 = [(0, 1, 2, 3), (4, 5, 6, 7), (8, 9)]      # round 1 (128-wide)
            PC2 = [(8, 9), (0, 1, 2, 3), (4, 5, 6, 7)]      # round 2 (by C pass)

            def mk_ag(tag, pieces, w):
                ins = [dram.tile([len(ms) * P, w], BF16, name=f"agi{tag}_{i}")
                       for i, ms in enumerate(pieces)]
                outs = [dram.tile([CORES * len(ms) * P, w], BF16,
                                  name=f"ago{tag}_{i}", addr_space="Shared")
                        for i, ms in enumerate(pieces)]
                return ins, outs

            agi0_t, ago0_t = mk_ag("0", PC0, F)
            agi1_t, ago1_t = mk_ag("1", PC1, OUT)
            agi2_t, ago2_t = mk_ag("2", PC2, OUT)

            def piece_of(pieces, mb):
                for i, ms in enumerate(pieces):
                    if mb in ms:
                        return i, ms.index(mb)
                raise AssertionError

            def stage_piece(agi_list, pieces, mb, src_ap):
                i, idx = piece_of(pieces, mb)
                nc.sync.dma_start(agi_list[i][idx * P:(idx + 1) * P, :], src_ap)

            def emit_ag(iap, oap):
                nc.gpsimd.collective_compute(
                    "AllGather", mybir.AluOpType.bypass,
                    replica_groups=[list(range(CORES))],
                    ins=[iap.opt()], outs=[oap.opt()],
                )

            def reload_piece(ago_list, pieces, i, dst_set, w):
                ms = pieces[i]
                for c8 in range(CORES):
                    for idx, m in enumerate(ms):
                        kt = c8 * KPC + m
                        if kt >= KTE:
                            continue
                        base = (c8 * len(ms) + idx) * P
                        nc.sync.dma_start(dst_set[kt][:, :w],
                                          ago_list[i][base:base + P, :])

            def stage_s1(mb, pp_ap):
                sc = wrk.tile([P, F], BF16, tag="sc", bufs=2, name=f"scA_{mb}")
                nc.vector.tensor_scalar_mul(sc[:], pp_ap, dd[:, mb:mb + 1])
                nc.vector.tensor_add(sc[:], sc[:], d1z_t[mb][:])
                stage_piece(agi0_t, PC0, mb, sc[:])

            for mb in range(W1MB):
                stage_s1(mb, pp_w1[mb][:, :F])
            emit_ag(agi0_t[0][:], ago0_t[0][:])
            reload_piece(ago0_t, PC0, 0, uB, F)

            # ---- hop A wave 2 (mbs 5-9), piece AGs as their mb-groups close ----
            for mb in range(W1MB, MB):
                pp = pprop.tile([P, 512], F32, tag="pp", name=f"ppw2_{mb}")
                for kt in range(KTE):
                    nc.tensor.matmul(
                        pp[:, :F], at_sl(kt, mb * P, P), uA[kt][:],
                        start=(kt == 0), stop=(kt == KTE - 1),
                    )
                stage_s1(mb, pp[:, :F])
                if mb == 7:
                    emit_ag(agi0_t[1][:], ago0_t[1][:])
                    reload_piece(ago0_t, PC0, 1, uB, F)
                elif mb == 9:
                    emit_ag(agi0_t[2][:], ago0_t[2][:])
                    reload_piece(ago0_t, PC0, 2, uB, F)

            # ---- hop B + layer-2 feature matmuls, software-pipelined ----
            # kt consumption order matches round-0 piece arrival
            orderB = [c8 * KPC + m for ms in PC0 for m in ms for c8 in range(CORES)]
            orderB = [kt for kt in orderB if kt < KTE]

            pp_b = [None] * MB

            def post_B(m):
                h = wrk.tile([P, F], BF16, tag="h", bufs=2, name=f"h_{m}")
                nc.vector.tensor_scalar_mul(h[:], pp_b[m][:, :F], ndiso[:, m:m + 1])
                nc.vector.tensor_add(h[:], h[:], e0_t[m][:])
                nc.vector.tensor_add(h[:], h[:], b1r[:])
                nc.vector.tensor_scalar_max(h[:], h[:], 0.0)
                hT = []
                for c in range(2):
                    tps = ptr.tile([P, P], BF16, tag="tr", name=f"hTp_{m}_{c}")
                    nc.tensor.transpose(tps[:], h[:, c * P:(c + 1) * P], idb[:])
                    tb = res.tile([P, HID], BF16, tag="wh", bufs=6, name=f"hTs_{m}_{c}")
                    nc.vector.tensor_copy(tb[:, :P], tps[:])
                    hT.append(tb[:, :P])
                zp = pterm.tile([P, F], F32, tag="tp", name=f"z1p_{m}")
                mm6(zp[:, :OUT], hT, w2t[1])
                nc.vector.tensor_scalar_mul(d1z_t[m][:, :OUT], zp[:, :OUT],
                                            diso[:, m:m + 1])
                z2p = pterm.tile([P, F], F32, tag="tp", name=f"z2p_{m}")
                mm6(z2p[:, :OUT], hT, w2t[2])
                sc = wrk.tile([P, F], BF16, tag="sc", bufs=2, name=f"scB_{m}")
                nc.vector.tensor_scalar_mul(sc[:, :OUT], z2p[:, :OUT], diso[:, m:m + 1])
                stage_piece(agi1_t, PC1, m, sc[:, :OUT])
                hwp = pterm.tile([P, F], F32, tag="tp", name=f"hwp_{m}")
                mm6(hwp[:, :OUT], hT, w2t[0])
                nc.vector.tensor_copy(hw_t[m][:, :OUT], hwp[:, :OUT])
                for i, ms in enumerate(PC1):
                    if m == ms[-1]:
                        emit_ag(agi1_t[i][:], ago1_t[i][:])
                        reload_piece(ago1_t, PC1, i, uA, OUT)

            for mb in range(5):
                pp_b[mb] = pprop.tile([P, 512], F32, tag="pp", name=f"ppb_{mb}")
            for kt in orderB:
                for mb in range(5):
                    nc.tensor.matmul(
                        pp_b[mb][:, :F], at_sl(kt, mb * P, P), uB[kt][:],
                        start=(kt == orderB[0]), stop=(kt == orderB[-1]),
                    )
            for mb in range(5):
                post_B(mb)
            for mb in range(5, MB + 1):
                if mb < MB:
                    pp_b[mb] = pprop.tile([P, 512], F32, tag="pp", name=f"ppb_{mb}")
                    for kt in orderB:
                        nc.tensor.matmul(
                            pp_b[mb][:, :F], at_sl(kt, mb * P, P), uB[kt][:],
                            start=(kt == orderB[0]), stop=(kt == orderB[-1]),
                        )
                if mb > 5:
                    post_B(mb - 1)

            # ---- hops C and D: transposed, one pass per dst chunk so staging
            # is staggered. 256-wide chunk first in C (earliest AG2 piece);
            # 512-wide chunks first in D (256 chunk's input arrives last).
            passC = [(1024, 256), (0, 512), (512, 512)]
            passD = [(0, 512), (512, 512), (1024, 256)]
            orderC = [c8 * KPC + m for ms in PC1 for m in ms for c8 in range(CORES)]
            orderC = [kt for kt in orderC if kt < KTE]
            orderD = [c8 * KPC + m for ms in PC2 for m in ms for c8 in range(CORES)]
            orderD = [kt for kt in orderD if kt < KTE]

            ppc = [None] * 3

            def prop_T(pi, off, w, u_set, order, tagn):
                ppc[pi] = pprop.tile([P, 512], F32, tag="pp", name=f"{tagn}_{pi}")
                for j, kt in enumerate(order):
                    nc.tensor.matmul(
                        ppc[pi][:, :w], u_set[kt][:, :OUT], at_sl(kt, off, w),
                        start=(j == 0), stop=(j == len(order) - 1),
                    )

            def post_C(pi):
                off, w = passC[pi]
                sT = res.tile([P, 512], F32, tag="xoTsT", bufs=2, name=f"sTc_{pi}")
                nc.vector.tensor_copy(sT[:, :w], ppc[pi][:, :w])
                for i in range(w // P):
                    mb = off // P + i
                    tps = ptr.tile([P, P], F32, tag="tr", name=f"trC_{mb}")
                    nc.tensor.transpose(tps[:], sT[:, i * P:(i + 1) * P], ident[:])
                    sc = wrk.tile([P, F], BF16, tag="sc", bufs=2, name=f"scC_{mb}")
                    nc.vector.tensor_scalar_mul(sc[:, :OUT], tps[:], dd[:, mb:mb + 1])
                    nc.vector.tensor_add(sc[:, :OUT], sc[:, :OUT], d1z_t[mb][:, :OUT])
                    stage_piece(agi2_t, PC2, mb, sc[:, :OUT])
                    for i2, ms in enumerate(PC2):
                        if mb == ms[-1]:
                            emit_ag(agi2_t[i2][:], ago2_t[i2][:])
                            reload_piece(ago2_t, PC2, i2, uB, OUT)

            for pi in range(3):
                off, w = passC[pi]
                prop_T(pi, off, w, uA, orderC, "ppc")
                post_C(pi)

            def post_D(pi):
                off, w = passD[pi]
                sT = res.tile([P, 512], F32, tag="xoTsT", bufs=2, name=f"sTd_{pi}")
                nc.vector.tensor_copy(sT[:, :w], ppc[pi][:, :w])
                for i in range(w // P):
                    mb = off // P + i
                    tps = ptr.tile([P, P], F32, tag="tr", name=f"trD_{mb}")
                    nc.tensor.transpose(tps[:], sT[:, i * P:(i + 1) * P], ident[:])
                    oacc = wrk.tile([P, OUT], F32, tag="sf", bufs=2, name=f"oacc_{mb}")
                    nc.vector.tensor_scalar_mul(oacc[:], tps[:], ndiso[:, mb:mb + 1])
                    nc.vector.tensor_add(oacc[:], oacc[:], hw_t[mb][:, :OUT])
                    nc.vector.tensor_add(oacc[:], oacc[:], b2r[:])
                    nc.sync.dma_start(out_d[mb * P:(mb + 1) * P, :], oacc[:])

            ppd = [pprop.tile([P, 512], F32, tag="pp", name=f"ppd_{pi}")
                   for pi in range(3)]
            for pi in range(3):
                ppc[pi] = ppd[pi]
            nD = len(orderD)
            for j, kt in enumerate(orderD):
                for pi in range(3):
                    off, w = passD[pi]
                    nc.tensor.matmul(
                        ppd[pi][:, :w], uB[kt][:, :OUT], at_sl(kt, off, w),
                        start=(j == 0), stop=(j == nD - 1),
                    )
            for pi in range(3):
                post_D(pi)

    nc.compile()
    return nc


def _prepare_inputs(x, edge, W1, b1, W2, b2):
    x = np.asarray(x, np.float32)
    edge = np.asarray(edge)
    W1 = np.asarray(W1, np.float32)
    b1 = np.asarray(b1, np.float32)
    W2 = np.asarray(W2, np.float32)
    b2 = np.asarray(b2, np.float32)
    src = edge[0].astype(np.int64)
    dst = edge[1].astype(np.int64)

    deg = np.bincount(dst, minlength=N).astype(np.float32)
    dis = np.where(deg > 0, 1.0 / np.sqrt(np.maximum(deg, 1.0)), 0.0).astype(np.float32)

    # dense transposed adjacency counts AT[s, d]
    flat = src * NP + dst
    uniq, cnt = np.unique(flat, return_counts=True)
    at8 = np.zeros(NP * NP, dtype=ml_dtypes.float8_e4m3)
    at8[uniq] = cnt.astype(ml_dtypes.float8_e4m3)
    at8 = at8.reshape(NP, NP)

    dis_pad = np.zeros(NP, np.float32)
    dis_pad[:N] = dis
    x_pad = np.zeros((NP, F), np.float32)
    x_pad[:N] = x

    w1x = np.stack([W1[0] - W1[2], W1[1], 2.0 * W1[2]]).astype(ml_dtypes.bfloat16)
    w2x = np.stack([W2[0] - W2[2], W2[1], 2.0 * W2[2]]).astype(ml_dtypes.bfloat16)
    b1r = np.broadcast_to(b1, (P, HID)).astype(ml_dtypes.bfloat16).copy()
    b2r = np.broadcast_to(b2, (P, OUT)).astype(ml_dtypes.bfloat16).copy()

    xTb = np.ascontiguousarray(x_pad.T).astype(ml_dtypes.bfloat16).reshape(2, P, NP)
    disf_h = np.ascontiguousarray(dis_pad.reshape(KT, P).T)
    in_maps = []
    for c in range(CORES):
        rows = slice(c * RPC, (c + 1) * RPC)
        dv = dis_pad[rows]
        atc = np.ascontiguousarray(
            at8[:, rows].reshape(KT, P, RPC).transpose(1, 0, 2).reshape(P, KT * RPC))
        m = {
            "at": atc,
            "xoT": np.ascontiguousarray(x_pad[rows].T).astype(ml_dtypes.bfloat16),
            "xT": xTb,
            "disf": disf_h,
            "diso": np.ascontiguousarray(dv.reshape(MB, P).T),
            "ndiso": np.ascontiguousarray((-dv).reshape(MB, P).T),
            "dd": np.ascontiguousarray((-dv * dv).reshape(MB, P).T),
            "w1x": w1x,
            "w2x": w2x,
            "b1r": b1r,
            "b2r": b2r,
        }
        in_maps.append(m)
    return in_maps


def _run(in_maps, trace=False, **kw):
    if "nc" not in _STATE:
        _STATE["nc"] = _build()
    r = run_bass_kernel_spmd(_STATE["nc"], in_maps, core_ids=list(range(CORES)),
                             trace=trace, **kw)
    out = np.concatenate([r.results[c]["outo"] for c in range(CORES)], axis=0)
    return out[:N], r


def kernel(**inputs) -> np.ndarray:
    in_maps = _prepare_inputs(**inputs)
    out, _ = _run(in_maps)
    return out
